# revision 2
# baseline (speedup 1.0000x reference)
"""ChannelPatchEmbed kernel for Trainium2 (8 NeuronCores, batch-parallel).

Computation: concat 8 single-feature channels -> each 512x512 image goes
through the SAME 1->96 conv (4x4 patches, stride 4) + bias.
Output: [8, 768, 128, 128] f32.

Strategy per core (1 batch sample per core):
  - The conv is a GEMM: for each patch, contract its 16 pixels against
    W[96, 16].  We pack all 8 channels x 16 patch-pixels onto the
    128-partition contraction dim (K = (c, i, j) = 8*4*4 = 128) and use a
    host-precomputed block-diagonal stationary matrix S so that one K=128
    matmul computes 16 output channels for all 8 input channels at once.
    6 such "oc chunk" matmuls cover all 96 output channels.
  - Input rows are DMA'd contiguously; the intra-row shift j is baked into
    the DMA (partition (c,i,j) holds image rows of channel c, row-offset i,
    pre-shifted left by j).  The matmul rhs then reads with a uniform
    stride-4 access pattern.
  - Bias is fused into the PSUM->SBUF eviction (ACT/DVE alternating).
"""

import contextlib
import sys

import numpy as np

if "/opt/trn_rl_repo" not in sys.path:
    sys.path.insert(0, "/opt/trn_rl_repo")

import concourse.bacc as bacc
import concourse.bass as bass
import concourse.mybir as mybir
import concourse.tile as tile
from concourse.bass_utils import run_bass_kernel_spmd

F32 = mybir.dt.float32

N_CORES = 8
C = 8            # input channels per sample (3 rgb + 4 hs + 1 dem)
H = 512          # image height/width
PATCH = 4
HP = H // PATCH  # 128 patches per side
EMBED = 96
CHUNKS = 6       # 96 output channels in chunks of 16
OCP = 16         # output channels per chunk
HPAD = 516       # padded image rows (see kernel())
HB = 16          # patch-rows per block
NBLK = HP // HB  # 8 blocks
WIN = 4          # windows per block (each window = 4 patch rows = N=512 cols)

_NC_CACHE = None


def _build_nc(reps=1, hw_loop=1):
    # detect_race_conditions=False: the sim race detector resolves SBUF APs to
    # a flat base+partition*row_bytes address model, which false-positives on
    # any concurrently-written partition-strided tiles (e.g. double-buffered
    # block N / block N+1 input loads in disjoint pool slots).
    # Bacc (not plain Bass): its compile() pipeline legalizes sync waits to
    # the HW limit of 1 per instruction (generate_event_semaphores) and moves
    # matmul waits onto ldweights.
    # reps>1 repeats the whole kernel body (identical work) for differential
    # wall-clock timing on hardware.
    nc = bacc.Bacc("TRN2", target_bir_lowering=False, detect_race_conditions=False)
    # x is host-padded to 516 rows so the j-shifted full-512 row reads stay
    # in-bounds (they read up to 3 elems past a row end, and up to 4 rows +
    # 3 elems past the last image row of a channel).
    x = nc.dram_tensor("x", [C, HPAD, H], F32, kind="ExternalInput")
    s = nc.dram_tensor("s", [CHUNKS, 128, 128], F32, kind="ExternalInput")
    bias = nc.dram_tensor("bias", [128, 128], F32, kind="ExternalInput")
    y = nc.dram_tensor("y", [C * EMBED, HP, HP], F32, kind="ExternalOutput")

    y_v = y.rearrange("ch h w -> ch (h w)")  # [768, 16384]

    with tile.TileContext(nc) as tc:
        with (
            tc.tile_pool(name="const", bufs=1) as const_pool,
            tc.tile_pool(name="rin", bufs=2) as r_pool,
            tc.tile_pool(name="stage", bufs=8) as stage_pool,
            tc.tile_pool(name="psum", bufs=8, space="PSUM") as psum_pool,
        ):
            # Pad so every subsequent tile is 512 B-aligned: the framework's
            # const-scalar region ends at +128 B, and the race detector (and
            # SDMA's sub-512B RMW path) works on 512 B granules — cross-tensor
            # granule sharing between DMA writers would be flagged as a race.
            _align_pad = const_pool.tile([128, 96], F32, tag="align_pad")
            # Stationary block-diag weights: s_sb[p, chunk*128 + m]
            s_sb = const_pool.tile([128, CHUNKS * 128], F32)
            nc.sync.dma_start(
                out=s_sb[:].rearrange("p (k m) -> p k m", k=CHUNKS),
                in_=s.rearrange("k p m -> p k m"),
            )
            # Bias: bias_sb[p, chunk] (padded to 512 B/partition so the DMA's
            # sub-512B RMW write can't share a granule with the next tile)
            bias_sb = const_pool.tile([128, 128], F32)
            nc.sync.dma_start(out=bias_sb[:], in_=bias[:])

            loop_cm = (
                tc.For_i(0, hw_loop, 1) if hw_loop > 1 else contextlib.nullcontext()
            )
            with loop_cm:
                for blk in range(NBLK * reps):
                    blk = blk % NBLK
                    hp0 = HB * blk
                    r0 = PATCH * hp0  # first image row of this block

                    # R: partition p = 8*(4i+j) + c holds, for each of the
                    # block's 16 patch-rows hl, image row r0+4hl+i of channel c
                    # shifted left by j (so free pos hl*512 + m = x[c, r0+4hl+i, m+j]).
                    # (i,j)-major so each DMA writes a CONTIGUOUS 8-partition
                    # slice — the sim's shadow memory mis-tracks partition-strided
                    # DMA writes.
                    R = r_pool.tile([128, HB * H], F32)
                    xf = x.rearrange("c r m -> c (r m)")  # [8, HPAD*512]
                    for i in range(PATCH):
                        for j in range(PATCH):
                            # rows r0+4*hl+i of every channel, shifted left by j.
                            # Full 512-elem chunks: reads cross row ends by up to
                            # j elems (harmless junk, those positions are never
                            # consumed by the matmul; host pad keeps it in-bounds).
                            off = (r0 + i) * H + j
                            src = xf[:, off : off + HB * PATCH * H].rearrange(
                                "c (hl m) -> c hl m", m=PATCH * H
                            )[:, :, :H]  # [8, 16, 512]
                            g = C * (PATCH * i + j)
                            dst = R[g : g + C].rearrange(
                                "c (hl m) -> c hl m", m=H
                            )  # [8, 16, 512]
                            nc.sync.dma_start(out=dst, in_=src)

                    for chunk in range(CHUNKS):
                        lhsT = s_sb[:, chunk * 128 : (chunk + 1) * 128]
                        stg = stage_pool.tile([128, WIN * 512], F32)
                        for w in range(WIN):
                            ps = psum_pool.tile([128, 512], F32)
                            rhs = R[:, w * 2048 : (w + 1) * 2048 : PATCH]  # [128, 512]
                            nc.tensor.matmul(ps[:], lhsT, rhs, start=True, stop=True)
                            out_sl = stg[:, w * 512 : (w + 1) * 512]
                            if w % 2 == 0:
                                nc.scalar.activation(
                                    out_sl,
                                    ps[:],
                                    mybir.ActivationFunctionType.Identity,
                                    bias=bias_sb[:, chunk : chunk + 1],
                                )
                            else:
                                nc.vector.tensor_scalar_add(
                                    out_sl, ps[:], bias_sb[:, chunk : chunk + 1]
                                )
                        # stg partition p -> y channel 128*chunk + p (affine!)
                        # Single full-width 128-partition 1 MB DMA, plain 2D
                        # AP (no partition-dim split).
                        # nc.scalar = the second HWDGE ring (qActDynamicHW):
                        # input loads go on the SP ring, stores on the ACT
                        # ring, so the two directions don't serialize on one
                        # descriptor ring.
                        nc.scalar.dma_start(
                            out=y_v[
                                128 * chunk : 128 * (chunk + 1),
                                hp0 * HP : (hp0 + HB) * HP,
                            ],
                            in_=stg[:],
                        )
    nc.compile()
    return nc


def _get_nc():
    global _NC_CACHE
    if _NC_CACHE is None:
        _NC_CACHE = _build_nc()
    return _NC_CACHE


def _host_prep(W, b):
    # Stationary chunk t computes GLOBAL output channels g = 128t + m
    # (m = psum partition).  g maps to input channel c = g//96 and conv
    # output channel oc = g%96, so psum partition <-> y channel is affine
    # and the store DMA is a full-width 128-partition transfer.
    # S[t, 8*(4i+j)+c(g), m] = W[oc(g), 0, i, j]
    W2 = np.ascontiguousarray(W, dtype=np.float32).reshape(EMBED, 16)  # [oc, kij]
    S = np.zeros((CHUNKS, 128, 128), np.float32)
    kij = np.arange(16)
    m = np.arange(128)
    for t in range(CHUNKS):
        g = 128 * t + m
        c = g // EMBED
        oc = g % EMBED
        S[t][8 * kij[:, None] + c[None, :], m[None, :]] = W2[oc, :].T
    b = np.asarray(b, dtype=np.float32)
    # bias_pad[p, t] = b[(128t+p) % 96]  (padded to [128, 128])
    bias_pad = np.zeros((128, 128), np.float32)
    for t in range(CHUNKS):
        bias_pad[:, t] = b[(128 * t + m) % EMBED]
    return S, bias_pad


def _timing_setup(inputs):
    """Build (nc, in_maps) exactly as kernel() would — for test.py --time."""
    rgb, hs, dem, W, b = (inputs[k] for k in ("rgb", "hs", "dem", "W", "b"))
    x_pad = np.zeros((N_CORES, C, HPAD, H), np.float32)
    x_pad[:, :3, :H] = np.asarray(rgb)
    x_pad[:, 3:7, :H] = np.asarray(hs)
    x_pad[:, 7:, :H] = np.asarray(dem)
    S, bias_mat = _host_prep(W, b)
    nc = _get_nc()
    in_maps = [
        {"x": x_pad[core], "s": S, "bias": bias_mat} for core in range(N_CORES)
    ]
    return nc, in_maps


def kernel(rgb, hs, dem, W, b):
    x_pad = np.zeros((N_CORES, C, HPAD, H), np.float32)
    x_pad[:, :3, :H] = np.asarray(rgb)
    x_pad[:, 3:7, :H] = np.asarray(hs)
    x_pad[:, 7:, :H] = np.asarray(dem)
    S, bias_mat = _host_prep(W, b)

    nc = _get_nc()
    in_maps = [
        {"x": x_pad[core], "s": S, "bias": bias_mat} for core in range(N_CORES)
    ]
    res = run_bass_kernel_spmd(nc, in_maps, list(range(N_CORES)))
    return np.stack([res.results[core]["y"] for core in range(N_CORES)], axis=0)



# revision 6
# speedup vs baseline: 3.3110x; 3.3110x over previous
"""ChannelPatchEmbed kernel for Trainium2 (8 NeuronCores, batch-parallel).

Computation: concat 8 single-feature channels -> each 512x512 image goes
through the SAME 1->96 conv (4x4 patches, stride 4) + bias.
Output: [8, 768, 128, 128] f32.

Strategy per core (1 batch sample per core):
  - GEMM formulation: K = (j, i, c) = 4*4*8 = 128 on the contraction
    partitions, block-diagonal stationary S (6 chunks of 16 output
    channels x 8 input channels = 128 M) -> one K=128 matmul yields 128
    output channels per 512-column pass.
  - bf16: inputs and W are host-cast to bf16 (tolerance 2e-2, bf16
    error ~4e-4); PE runs at full rate, PSUM accumulates f32.
  - Input rows are loaded from HBM exactly ONCE per block as a single
    32-partition DMA (partition (i,c) holds rows r0+4hl+i); the three
    j-shifted copies the matmul layout needs are built on-chip with
    SBUF->SBUF DMAs (32 x 16KB descriptors, cheap) instead of re-reading
    HBM 4x in 2KB packets like the old version.
  - Bias is fused into the PSUM->SBUF eviction (ACT/DVE alternating).
"""

import sys

import numpy as np

if "/opt/trn_rl_repo" not in sys.path:
    sys.path.insert(0, "/opt/trn_rl_repo")

import ml_dtypes

import concourse.bacc as bacc
import concourse.mybir as mybir
import concourse.tile as tile
from concourse.bass_utils import run_bass_kernel_spmd

F32 = mybir.dt.float32
BF16 = mybir.dt.bfloat16

N_CORES = 8
C = 8            # input channels per sample (3 rgb + 4 hs + 1 dem)
H = 512          # image height/width
PATCH = 4
HP = H // PATCH  # 128 patches per side
EMBED = 96
CHUNKS = 6       # 96*8 = 768 output channels in chunks of 128
HB = 16          # patch-rows per block
NBLK = HP // HB  # 8 blocks
WIN = 4          # windows per block (each window = 4 patch rows = N=512 cols)
RFREE = HB * H   # used free elems per R partition (8192)
RPAD = RFREE + 256  # slack so j-shifted SBUF reads stay in-tile; 16896B = 33*512

_NC_CACHE = None


def _build_nc():
    # detect_race_conditions=False: the sim race detector resolves SBUF APs to
    # a flat base+partition*row_bytes address model, which false-positives on
    # concurrently-accessed partition-sliced tiles (e.g. the j-shift copies
    # that read partitions 0-31 of R while writing partitions 32j..32j+31).
    nc = bacc.Bacc("TRN2", target_bir_lowering=False, detect_race_conditions=False)
    # x is host-pre-shuffled to [i, c, patch-row h, m]: x[i, c, h, :] =
    # image[c, 4h+i, :].  Each (i, c, 16-patch-row block) is then a single
    # CONTIGUOUS 16 KB run in HBM -> line-rate load descriptors.
    x = nc.dram_tensor("x", [PATCH, C, HP, H], BF16, kind="ExternalInput")
    s = nc.dram_tensor("s", [CHUNKS, 128, 128], BF16, kind="ExternalInput")
    bias = nc.dram_tensor("bias", [128, 128], F32, kind="ExternalInput")
    y = nc.dram_tensor("y", [C * EMBED, HP, HP], F32, kind="ExternalOutput")

    y_v = y.rearrange("ch h w -> ch (h w)")  # [768, 16384]
    # partition (i c), free = (global patch-row h, column m): row 4h+i
    x_ic = x.rearrange("i c h m -> (i c) h m")  # [32, 128, 512]

    with tile.TileContext(nc) as tc:
        with (
            tc.tile_pool(name="const", bufs=1) as const_pool,
            tc.tile_pool(name="rin", bufs=2) as r_pool,
            tc.tile_pool(name="stage", bufs=8) as stage_pool,
            tc.tile_pool(name="psum", bufs=8, space="PSUM") as psum_pool,
        ):
            # Pad so every subsequent tile is 512 B-aligned: the framework's
            # const-scalar region ends at +128 B, and SDMA's sub-512B write
            # path does RMW on 512 B granules — cross-tensor granule sharing
            # between concurrent DMA writers would corrupt data.
            _align_pad = const_pool.tile([128, 96], F32, tag="align_pad")
            # Stationary block-diag weights: s_sb[p, chunk*128 + m], bf16,
            # padded to 2048 B/partition.
            s_sb = const_pool.tile([128, 1024], BF16)
            nc.sync.dma_start(
                out=s_sb[:, : CHUNKS * 128].rearrange("p (k m) -> p k m", k=CHUNKS),
                in_=s.rearrange("k p m -> p k m"),
            )
            # Bias: bias_sb[p, chunk] (512 B/partition)
            bias_sb = const_pool.tile([128, 128], F32)
            nc.sync.dma_start(out=bias_sb[:], in_=bias[:])

            for blk in range(NBLK):
                hp0 = HB * blk

                # R: partition p = 32j + 8i + c holds, for each of the block's
                # 16 patch-rows hl, image row 4*(hp0+hl)+i of channel c
                # shifted left by j (free pos hl*512 + m = x[c, row, m+j]).
                R = r_pool.tile([128, RPAD], BF16)
                # j=0 slice comes straight from HBM: one 32-partition DMA,
                # 16 row-descriptors of 1 KB per partition.
                nc.sync.dma_start(
                    out=R[0:32, 0:RFREE].rearrange("p (h m) -> p h m", m=H),
                    in_=x_ic[:, hp0 : hp0 + HB, :],
                )
                # j=1..3 are on-chip shifted copies of the j=0 slice:
                # 32 contiguous ~16KB descriptors each, partition-offset +32j,
                # element-offset +j.  SWDGE (gpsimd) keeps them off the two
                # HWDGE rings that carry the loads and stores.
                for j in range(1, PATCH):
                    nc.gpsimd.dma_start(
                        out=R[32 * j : 32 * j + 32, 0:RFREE],
                        in_=R[0:32, j : j + RFREE],
                    )

                for chunk in range(CHUNKS):
                    lhsT = s_sb[:, chunk * 128 : (chunk + 1) * 128]
                    stg = stage_pool.tile([128, WIN * 512], F32)
                    for w in range(WIN):
                        ps = psum_pool.tile([128, 512], F32)
                        rhs = R[:, w * 2048 : (w + 1) * 2048 : PATCH]  # [128, 512]
                        nc.tensor.matmul(ps[:], lhsT, rhs, start=True, stop=True)
                        out_sl = stg[:, w * 512 : (w + 1) * 512]
                        if w % 2 == 0:
                            nc.scalar.activation(
                                out_sl,
                                ps[:],
                                mybir.ActivationFunctionType.Identity,
                                bias=bias_sb[:, chunk : chunk + 1],
                            )
                        else:
                            nc.vector.tensor_scalar_add(
                                out_sl, ps[:], bias_sb[:, chunk : chunk + 1]
                            )
                    # stg partition p -> y channel 128*chunk + p (affine!)
                    # Single full-width 128-partition 1 MB DMA on the ACT
                    # HWDGE ring so loads (SP ring) and stores don't
                    # serialize on one descriptor ring.
                    nc.scalar.dma_start(
                        out=y_v[
                            128 * chunk : 128 * (chunk + 1),
                            hp0 * HP : (hp0 + HB) * HP,
                        ],
                        in_=stg[:],
                    )
    nc.compile()
    return nc


def _get_nc():
    global _NC_CACHE
    if _NC_CACHE is None:
        _NC_CACHE = _build_nc()
    return _NC_CACHE


def _host_prep(W, b):
    # Stationary chunk t computes GLOBAL output channels g = 128t + m
    # (m = psum partition).  g maps to input channel c = g//96 and conv
    # output channel oc = g%96, so psum partition <-> y channel is affine
    # and the store DMA is a full-width 128-partition transfer.
    # K index k = 32j + 8i + c:  S[t, k, m] = W[oc(g), 0, i, j]
    W2 = np.ascontiguousarray(W, dtype=np.float32).reshape(EMBED, PATCH, PATCH)
    S = np.zeros((CHUNKS, 128, 128), np.float32)
    m = np.arange(128)
    for t in range(CHUNKS):
        g = 128 * t + m
        c = g // EMBED
        oc = g % EMBED
        for i in range(PATCH):
            for j in range(PATCH):
                S[t][32 * j + 8 * i + c, m] = W2[oc, i, j]
    b = np.asarray(b, dtype=np.float32)
    # bias_pad[p, t] = b[(128t+p) % 96]  (padded to [128, 128])
    bias_pad = np.zeros((128, 128), np.float32)
    for t in range(CHUNKS):
        bias_pad[:, t] = b[(128 * t + m) % EMBED]
    return S.astype(ml_dtypes.bfloat16), bias_pad


def _prep_inputs(rgb, hs, dem, W, b):
    x16 = np.empty((N_CORES, C, H, H), ml_dtypes.bfloat16)
    x16[:, :3] = np.asarray(rgb)
    x16[:, 3:7] = np.asarray(hs)
    x16[:, 7:] = np.asarray(dem)
    # [core, c, 4h+i, m] -> [core, i, c, h, m]
    x16 = np.ascontiguousarray(
        x16.reshape(N_CORES, C, HP, PATCH, H).transpose(0, 3, 1, 2, 4)
    )
    S, bias_mat = _host_prep(W, b)
    return [
        {"x": x16[core], "s": S, "bias": bias_mat} for core in range(N_CORES)
    ]


def _timing_setup(inputs):
    """Build (nc, in_maps) exactly as kernel() would — for test.py --time."""
    in_maps = _prep_inputs(
        inputs["rgb"], inputs["hs"], inputs["dem"], inputs["W"], inputs["b"]
    )
    return _get_nc(), in_maps


def kernel(rgb, hs, dem, W, b):
    in_maps = _prep_inputs(rgb, hs, dem, W, b)
    nc = _get_nc()
    res = run_bass_kernel_spmd(nc, in_maps, list(range(N_CORES)))
    return np.stack([res.results[core]["y"] for core in range(N_CORES)], axis=0)


# revision 12
# speedup vs baseline: 3.4819x; 1.0516x over previous
"""ChannelPatchEmbed kernel for Trainium2 (8 NeuronCores, batch-parallel).

Computation: concat 8 single-feature channels -> each 512x512 image goes
through the SAME 1->96 conv (4x4 patches, stride 4) + bias.
Output: [8, 768, 128, 128] f32.

Strategy per core (1 batch sample per core):
  - GEMM formulation: K = (j, i, c) = 4*4*8 = 128 on the contraction
    partitions, block-diagonal stationary S (6 chunks of 16 output
    channels x 8 input channels = 128 M) -> one K=128 matmul yields 128
    output channels per 512-column pass.
  - bf16: inputs and W are host-cast to bf16 (tolerance 2e-2, bf16
    error ~4e-4); PE runs at full rate, PSUM accumulates f32.
  - Input rows are loaded from HBM exactly ONCE per block as a single
    32-partition DMA (partition (i,c) holds rows r0+4hl+i); the three
    j-shifted copies the matmul layout needs are built on-chip with
    SBUF->SBUF DMAs (32 x 16KB descriptors, cheap) instead of re-reading
    HBM 4x in 2KB packets like the old version.
  - Bias is fused into the PSUM->SBUF eviction (ACT/DVE alternating).
"""

import sys

import numpy as np

if "/opt/trn_rl_repo" not in sys.path:
    sys.path.insert(0, "/opt/trn_rl_repo")

import ml_dtypes

import concourse.bacc as bacc
import concourse.mybir as mybir
import concourse.tile as tile
from concourse.bass_utils import run_bass_kernel_spmd

F32 = mybir.dt.float32
BF16 = mybir.dt.bfloat16

N_CORES = 8
C = 8            # input channels per sample (3 rgb + 4 hs + 1 dem)
H = 512          # image height/width
PATCH = 4
HP = H // PATCH  # 128 patches per side
EMBED = 96
CHUNKS = 6       # 96*8 = 768 output channels in chunks of 128
HB = 16          # patch-rows per block
NBLK = HP // HB  # 8 blocks
WIN = 4          # windows per block (each window = 4 patch rows = N=512 cols)
RFREE = HB * H   # used free elems per R partition (8192)
RPAD = RFREE + 256  # slack so j-shifted SBUF reads stay in-tile; 16896B = 33*512
XPAD = HP * H + 8   # per-(i,c) elems in x, padded so shifted reads stay in-bounds

_NC_CACHE = None


def _build_nc():
    # detect_race_conditions=False: the sim race detector resolves SBUF APs to
    # a flat base+partition*row_bytes address model, which false-positives on
    # concurrently-accessed partition-sliced tiles (e.g. the j-shift copies
    # that read partitions 0-31 of R while writing partitions 32j..32j+31).
    nc = bacc.Bacc("TRN2", target_bir_lowering=False, detect_race_conditions=False)
    # x is host-pre-shuffled to [i, c, patch-row h * 512 + m]: x[i, c, h*512+m]
    # = image[c, 4h+i, m].  Each (i, c, 16-patch-row block) is then a single
    # CONTIGUOUS 16 KB run in HBM -> line-rate load descriptors, and the
    # j-shifted variant of a block is the same run offset by 2j bytes (the
    # +8-elem pad keeps the last block's shifted read in-bounds).
    x = nc.dram_tensor("x", [PATCH, C, XPAD], BF16, kind="ExternalInput")
    s = nc.dram_tensor("s", [CHUNKS, 128, 128], BF16, kind="ExternalInput")
    bias = nc.dram_tensor("bias", [128, 128], F32, kind="ExternalInput")
    y = nc.dram_tensor("y", [C * EMBED, HP, HP], F32, kind="ExternalOutput")

    y_v = y.rearrange("ch h w -> ch (h w)")  # [768, 16384]
    # partition (i c), free = patch-row h * 512 + column m: row 4h+i
    x_ic = x.rearrange("i c f -> (i c) f")  # [32, XPAD]

    with tile.TileContext(nc) as tc:
        with (
            tc.tile_pool(name="const", bufs=1) as const_pool,
            tc.tile_pool(name="rin", bufs=2) as r_pool,
            tc.tile_pool(name="stage", bufs=8) as stage_pool,
            tc.tile_pool(name="psum", bufs=2, space="PSUM") as psum_pool,
        ):
            # Pad so every subsequent tile is 512 B-aligned: the framework's
            # const-scalar region ends at +128 B, and SDMA's sub-512B write
            # path does RMW on 512 B granules — cross-tensor granule sharing
            # between concurrent DMA writers would corrupt data.
            _align_pad = const_pool.tile([128, 96], F32, tag="align_pad")
            # Stationary block-diag weights: s_sb[p, chunk*128 + m], bf16,
            # padded to 2048 B/partition.
            s_sb = const_pool.tile([128, 1024], BF16)
            nc.sync.dma_start(
                out=s_sb[:, : CHUNKS * 128].rearrange("p (k m) -> p k m", k=CHUNKS),
                in_=s.rearrange("k p m -> p k m"),
            )
            # Bias: bias_sb[p, chunk] (512 B/partition)
            bias_sb = const_pool.tile([128, 128], F32)
            nc.sync.dma_start(out=bias_sb[:], in_=bias[:])

            for blk in range(NBLK):
                hp0 = HB * blk

                # R: partition p = 32j + 8i + c holds, for each of the block's
                # 16 patch-rows hl, image row 4*(hp0+hl)+i of channel c
                # shifted left by j (free pos hl*512 + m = x[c, row, m+j]).
                R = r_pool.tile([128, RPAD], BF16)
                # j=0 and j=1 slices come straight from HBM: 32-partition DMAs
                # with ONE contiguous 16 KB descriptor per partition (the j=1
                # source run is just offset by 2 bytes).
                for j in range(2):
                    nc.sync.dma_start(
                        out=R[32 * j : 32 * j + 32, 0:RFREE],
                        in_=x_ic[:, hp0 * H + j : hp0 * H + j + RFREE],
                    )
                # j=2 / j=3 are DVE cross-quadrant copies of j=0 / j=1 shifted
                # by 2 elements (4 bytes, so the 2x/4x DVE streaming modes can
                # engage).  At nch=32 the DVE output crossbar can route bank 0
                # to any quadrant, so [0:32]->[64:96] etc. is a single copy.
                for j in (2, 3):
                    nc.vector.tensor_copy(
                        out=R[32 * j : 32 * j + 32, 0:RFREE],
                        in_=R[32 * (j - 2) : 32 * (j - 2) + 32, 2 : 2 + RFREE],
                    )

                for chunk in range(CHUNKS):
                    lhsT = s_sb[:, chunk * 128 : (chunk + 1) * 128]
                    stg = stage_pool.tile([128, WIN * 512], F32)
                    # One 4-bank PSUM tile per chunk: 4 matmuls fill it, ONE
                    # wide eviction drains it (amortizes the ~0.5us/instr
                    # ACT/DVE overhead over 2048 elements).
                    ps = psum_pool.tile([128, WIN * 512], F32)
                    for w in range(WIN):
                        rhs = R[:, w * 2048 : (w + 1) * 2048 : PATCH]  # [128, 512]
                        nc.tensor.matmul(
                            ps[:, w * 512 : (w + 1) * 512], lhsT, rhs,
                            start=True, stop=True,
                        )
                    if chunk % 2 == 0:
                        nc.scalar.activation(
                            stg[:],
                            ps[:],
                            mybir.ActivationFunctionType.Identity,
                            bias=bias_sb[:, chunk : chunk + 1],
                        )
                    else:
                        nc.vector.tensor_scalar_add(
                            stg[:], ps[:], bias_sb[:, chunk : chunk + 1]
                        )
                    # stg partition p -> y channel 128*chunk + p (affine!)
                    # Single full-width 128-partition 1 MB DMA on the ACT
                    # HWDGE ring so loads (SP ring) and stores don't
                    # serialize on one descriptor ring.
                    nc.scalar.dma_start(
                        out=y_v[
                            128 * chunk : 128 * (chunk + 1),
                            hp0 * HP : (hp0 + HB) * HP,
                        ],
                        in_=stg[:],
                    )
    nc.compile()
    return nc


def _get_nc():
    global _NC_CACHE
    if _NC_CACHE is None:
        _NC_CACHE = _build_nc()
    return _NC_CACHE


def _host_prep(W, b):
    # Stationary chunk t computes GLOBAL output channels g = 128t + m
    # (m = psum partition).  g maps to input channel c = g//96 and conv
    # output channel oc = g%96, so psum partition <-> y channel is affine
    # and the store DMA is a full-width 128-partition transfer.
    # K index k = 32j + 8i + c:  S[t, k, m] = W[oc(g), 0, i, j]
    W2 = np.ascontiguousarray(W, dtype=np.float32).reshape(EMBED, PATCH, PATCH)
    S = np.zeros((CHUNKS, 128, 128), np.float32)
    m = np.arange(128)
    for t in range(CHUNKS):
        g = 128 * t + m
        c = g // EMBED
        oc = g % EMBED
        for i in range(PATCH):
            for j in range(PATCH):
                S[t][32 * j + 8 * i + c, m] = W2[oc, i, j]
    b = np.asarray(b, dtype=np.float32)
    # bias_pad[p, t] = b[(128t+p) % 96]  (padded to [128, 128])
    bias_pad = np.zeros((128, 128), np.float32)
    for t in range(CHUNKS):
        bias_pad[:, t] = b[(128 * t + m) % EMBED]
    return S.astype(ml_dtypes.bfloat16), bias_pad


def _prep_inputs(rgb, hs, dem, W, b):
    x16 = np.empty((N_CORES, C, H, H), ml_dtypes.bfloat16)
    x16[:, :3] = np.asarray(rgb)
    x16[:, 3:7] = np.asarray(hs)
    x16[:, 7:] = np.asarray(dem)
    # [core, c, 4h+i, m] -> [core, i, c, h*512+m], pad each (i,c) run to XPAD
    xs = np.zeros((N_CORES, PATCH, C, XPAD), ml_dtypes.bfloat16)
    xs[..., : HP * H] = (
        x16.reshape(N_CORES, C, HP, PATCH, H)
        .transpose(0, 3, 1, 2, 4)
        .reshape(N_CORES, PATCH, C, HP * H)
    )
    S, bias_mat = _host_prep(W, b)
    return [
        {"x": xs[core], "s": S, "bias": bias_mat} for core in range(N_CORES)
    ]


def _timing_setup(inputs):
    """Build (nc, in_maps) exactly as kernel() would — for test.py --time."""
    in_maps = _prep_inputs(
        inputs["rgb"], inputs["hs"], inputs["dem"], inputs["W"], inputs["b"]
    )
    return _get_nc(), in_maps


def kernel(rgb, hs, dem, W, b):
    in_maps = _prep_inputs(rgb, hs, dem, W, b)
    nc = _get_nc()
    res = run_bass_kernel_spmd(nc, in_maps, list(range(N_CORES)))
    return np.stack([res.results[core]["y"] for core in range(N_CORES)], axis=0)


# revision 15
# speedup vs baseline: 3.6254x; 1.0412x over previous
"""ChannelPatchEmbed kernel for Trainium2 (8 NeuronCores, batch-parallel).

Computation: concat 8 single-feature channels -> each 512x512 image goes
through the SAME 1->96 conv (4x4 patches, stride 4) + bias.
Output: [8, 768, 128, 128] f32.

Strategy per core (1 batch sample per core):
  - GEMM formulation: K = (j, i, c) = 4*4*8 = 128 on the contraction
    partitions, block-diagonal stationary S (6 chunks of 16 output
    channels x 8 input channels = 128 M) -> one K=128 matmul yields 128
    output channels per 512-column pass.
  - bf16: inputs and W are host-cast to bf16 (tolerance 2e-2, bf16
    error ~4e-4); PE runs at full rate, PSUM accumulates f32.
  - Input rows are loaded from HBM exactly ONCE per block as a single
    32-partition DMA (partition (i,c) holds rows r0+4hl+i); the three
    j-shifted copies the matmul layout needs are built on-chip with
    SBUF->SBUF DMAs (32 x 16KB descriptors, cheap) instead of re-reading
    HBM 4x in 2KB packets like the old version.
  - Bias is fused into the PSUM->SBUF eviction (ACT/DVE alternating).
"""

import sys

import numpy as np

if "/opt/trn_rl_repo" not in sys.path:
    sys.path.insert(0, "/opt/trn_rl_repo")

import ml_dtypes

import concourse.bacc as bacc
import concourse.mybir as mybir
import concourse.tile as tile
from concourse.bass_utils import run_bass_kernel_spmd

F32 = mybir.dt.float32
BF16 = mybir.dt.bfloat16

N_CORES = 8
C = 8            # input channels per sample (3 rgb + 4 hs + 1 dem)
H = 512          # image height/width
PATCH = 4
HP = H // PATCH  # 128 patches per side
EMBED = 96
CHUNKS = 6       # 96*8 = 768 output channels in chunks of 128
HB = 32          # patch-rows per block
NBLK = HP // HB  # 4 blocks
WIN = 8          # windows per block (each window = 4 patch rows = N=512 cols)
RFREE = HB * H   # used free elems per R partition (8192)
RPAD = RFREE + 256  # slack so j-shifted SBUF reads stay in-tile; 16896B = 33*512
XPAD = HP * H + 8   # per-(i,c) elems in x, padded so shifted reads stay in-bounds

_NC_CACHE = None


def _build_nc():
    # detect_race_conditions=False: the sim race detector resolves SBUF APs to
    # a flat base+partition*row_bytes address model, which false-positives on
    # concurrently-accessed partition-sliced tiles (e.g. the j-shift copies
    # that read partitions 0-31 of R while writing partitions 32j..32j+31).
    nc = bacc.Bacc("TRN2", target_bir_lowering=False, detect_race_conditions=False)
    # x is host-pre-shuffled to [i, c, patch-row h * 512 + m]: x[i, c, h*512+m]
    # = image[c, 4h+i, m].  Each (i, c, 16-patch-row block) is then a single
    # CONTIGUOUS 16 KB run in HBM -> line-rate load descriptors, and the
    # j-shifted variant of a block is the same run offset by 2j bytes (the
    # +8-elem pad keeps the last block's shifted read in-bounds).
    x = nc.dram_tensor("x", [PATCH, C, XPAD], BF16, kind="ExternalInput")
    s = nc.dram_tensor("s", [CHUNKS, 128, 128], BF16, kind="ExternalInput")
    bias = nc.dram_tensor("bias", [128, 128], F32, kind="ExternalInput")
    y = nc.dram_tensor("y", [C * EMBED, HP, HP], F32, kind="ExternalOutput")

    y_v = y.rearrange("ch h w -> ch (h w)")  # [768, 16384]
    # partition (i c), free = patch-row h * 512 + column m: row 4h+i
    x_ic = x.rearrange("i c f -> (i c) f")  # [32, XPAD]

    with tile.TileContext(nc) as tc:
        with (
            tc.tile_pool(name="const", bufs=1) as const_pool,
            tc.tile_pool(name="rin", bufs=2) as r_pool,
            tc.tile_pool(name="stage", bufs=4) as stage_pool,
            tc.tile_pool(name="psum", bufs=2, space="PSUM") as psum_pool,
        ):
            # Pad so every subsequent tile is 512 B-aligned: the framework's
            # const-scalar region ends at +128 B, and SDMA's sub-512B write
            # path does RMW on 512 B granules — cross-tensor granule sharing
            # between concurrent DMA writers would corrupt data.
            _align_pad = const_pool.tile([128, 96], F32, tag="align_pad")
            # Stationary block-diag weights: s_sb[p, chunk*128 + m], bf16,
            # padded to 2048 B/partition.
            s_sb = const_pool.tile([128, 1024], BF16)
            nc.sync.dma_start(
                out=s_sb[:, : CHUNKS * 128].rearrange("p (k m) -> p k m", k=CHUNKS),
                in_=s.rearrange("k p m -> p k m"),
            )
            # Bias: bias_sb[p, chunk] (512 B/partition)
            bias_sb = const_pool.tile([128, 128], F32)
            nc.sync.dma_start(out=bias_sb[:], in_=bias[:])

            for blk in range(NBLK):
                hp0 = HB * blk

                # R: partition p = 32j + 8i + c holds, for each of the block's
                # 16 patch-rows hl, image row 4*(hp0+hl)+i of channel c
                # shifted left by j (free pos hl*512 + m = x[c, row, m+j]).
                R = r_pool.tile([128, RPAD], BF16)
                # j=0 and j=1 slices come straight from HBM: 32-partition DMAs
                # with ONE contiguous 16 KB descriptor per partition (the j=1
                # source run is just offset by 2 bytes).
                for j in range(2):
                    nc.sync.dma_start(
                        out=R[32 * j : 32 * j + 32, 0:RFREE],
                        in_=x_ic[:, hp0 * H + j : hp0 * H + j + RFREE],
                    )
                # j=2 / j=3 are DVE cross-quadrant copies of j=0 / j=1 shifted
                # by 2 elements (4 bytes, so the 2x/4x DVE streaming modes can
                # engage).  At nch=32 the DVE output crossbar can route bank 0
                # to any quadrant, so [0:32]->[64:96] etc. is a single copy.
                for j in (2, 3):
                    nc.vector.tensor_copy(
                        out=R[32 * j : 32 * j + 32, 0:RFREE],
                        in_=R[32 * (j - 2) : 32 * (j - 2) + 32, 2 : 2 + RFREE],
                    )

                for chunk in range(CHUNKS):
                    lhsT = s_sb[:, chunk * 128 : (chunk + 1) * 128]
                    stg = stage_pool.tile([128, WIN * 512], F32)
                    # 4-bank PSUM tiles: 4 matmuls fill one, ONE wide eviction
                    # drains it (amortizes the ~0.5us/instr ACT/DVE overhead
                    # over 2048 elements).
                    for half in range(WIN // 4):
                        ps = psum_pool.tile([128, 4 * 512], F32)
                        for w4 in range(4):
                            w = half * 4 + w4
                            rhs = R[:, w * 2048 : (w + 1) * 2048 : PATCH]
                            nc.tensor.matmul(
                                ps[:, w4 * 512 : (w4 + 1) * 512], lhsT, rhs,
                                start=True, stop=True,
                            )
                        out_sl = stg[:, half * 2048 : (half + 1) * 2048]
                        if (chunk + half) % 2 == 0:
                            nc.scalar.activation(
                                out_sl,
                                ps[:],
                                mybir.ActivationFunctionType.Identity,
                                bias=bias_sb[:, chunk : chunk + 1],
                            )
                        else:
                            nc.vector.tensor_scalar_add(
                                out_sl, ps[:], bias_sb[:, chunk : chunk + 1]
                            )
                    # stg partition p -> y channel 128*chunk + p (affine!)
                    # Single full-width 128-partition 1 MB DMA on the ACT
                    # HWDGE ring so loads (SP ring) and stores don't
                    # serialize on one descriptor ring.
                    nc.scalar.dma_start(
                        out=y_v[
                            128 * chunk : 128 * (chunk + 1),
                            hp0 * HP : (hp0 + HB) * HP,
                        ],
                        in_=stg[:],
                    )
    nc.compile()
    return nc


def _get_nc():
    global _NC_CACHE
    if _NC_CACHE is None:
        _NC_CACHE = _build_nc()
    return _NC_CACHE


def _host_prep(W, b):
    # Stationary chunk t computes GLOBAL output channels g = 128t + m
    # (m = psum partition).  g maps to input channel c = g//96 and conv
    # output channel oc = g%96, so psum partition <-> y channel is affine
    # and the store DMA is a full-width 128-partition transfer.
    # K index k = 32j + 8i + c:  S[t, k, m] = W[oc(g), 0, i, j]
    W2 = np.ascontiguousarray(W, dtype=np.float32).reshape(EMBED, PATCH, PATCH)
    S = np.zeros((CHUNKS, 128, 128), np.float32)
    m = np.arange(128)
    for t in range(CHUNKS):
        g = 128 * t + m
        c = g // EMBED
        oc = g % EMBED
        for i in range(PATCH):
            for j in range(PATCH):
                S[t][32 * j + 8 * i + c, m] = W2[oc, i, j]
    b = np.asarray(b, dtype=np.float32)
    # bias_pad[p, t] = b[(128t+p) % 96]  (padded to [128, 128])
    bias_pad = np.zeros((128, 128), np.float32)
    for t in range(CHUNKS):
        bias_pad[:, t] = b[(128 * t + m) % EMBED]
    return S.astype(ml_dtypes.bfloat16), bias_pad


def _prep_inputs(rgb, hs, dem, W, b):
    x16 = np.empty((N_CORES, C, H, H), ml_dtypes.bfloat16)
    x16[:, :3] = np.asarray(rgb)
    x16[:, 3:7] = np.asarray(hs)
    x16[:, 7:] = np.asarray(dem)
    # [core, c, 4h+i, m] -> [core, i, c, h*512+m], pad each (i,c) run to XPAD
    xs = np.zeros((N_CORES, PATCH, C, XPAD), ml_dtypes.bfloat16)
    xs[..., : HP * H] = (
        x16.reshape(N_CORES, C, HP, PATCH, H)
        .transpose(0, 3, 1, 2, 4)
        .reshape(N_CORES, PATCH, C, HP * H)
    )
    S, bias_mat = _host_prep(W, b)
    return [
        {"x": xs[core], "s": S, "bias": bias_mat} for core in range(N_CORES)
    ]


def _timing_setup(inputs):
    """Build (nc, in_maps) exactly as kernel() would — for test.py --time."""
    in_maps = _prep_inputs(
        inputs["rgb"], inputs["hs"], inputs["dem"], inputs["W"], inputs["b"]
    )
    return _get_nc(), in_maps


def kernel(rgb, hs, dem, W, b):
    in_maps = _prep_inputs(rgb, hs, dem, W, b)
    nc = _get_nc()
    res = run_bass_kernel_spmd(nc, in_maps, list(range(N_CORES)))
    return np.stack([res.results[core]["y"] for core in range(N_CORES)], axis=0)


# revision 18
# speedup vs baseline: 3.7221x; 1.0267x over previous
"""ChannelPatchEmbed kernel for Trainium2 (8 NeuronCores, batch-parallel).

Computation: concat 8 single-feature channels -> each 512x512 image goes
through the SAME 1->96 conv (4x4 patches, stride 4) + bias.
Output: [8, 768, 128, 128] f32.

Strategy per core (1 batch sample per core):
  - GEMM formulation: K = (j, i, c) = 4*4*8 = 128 on the contraction
    partitions, block-diagonal stationary S (6 chunks of 16 output
    channels x 8 input channels = 128 M) -> one K=128 matmul yields 128
    output channels per 512-column pass.
  - bf16: inputs and W are host-cast to bf16 (tolerance 2e-2, bf16
    error ~4e-4); PE runs at full rate, PSUM accumulates f32.
  - Input rows are loaded from HBM exactly ONCE per block as a single
    32-partition DMA (partition (i,c) holds rows r0+4hl+i); the three
    j-shifted copies the matmul layout needs are built on-chip with
    SBUF->SBUF DMAs (32 x 16KB descriptors, cheap) instead of re-reading
    HBM 4x in 2KB packets like the old version.
  - Bias is fused into the PSUM->SBUF eviction (ACT/DVE alternating).
"""

import sys

import numpy as np

if "/opt/trn_rl_repo" not in sys.path:
    sys.path.insert(0, "/opt/trn_rl_repo")

import ml_dtypes

import concourse.bacc as bacc
import concourse.mybir as mybir
import concourse.tile as tile
from concourse.bass_utils import run_bass_kernel_spmd

F32 = mybir.dt.float32
BF16 = mybir.dt.bfloat16

N_CORES = 8
C = 8            # input channels per sample (3 rgb + 4 hs + 1 dem)
H = 512          # image height/width
PATCH = 4
HP = H // PATCH  # 128 patches per side
EMBED = 96
CHUNKS = 6       # 96*8 = 768 output channels in chunks of 128
# patch-rows per block: small first blocks shorten the load->copy->matmul
# ramp before the first stores flow; 32-row blocks give 16 KB store
# descriptors for the bulk.
BLOCKS = [8, 24, 32, 32, 32]
HBMAX = max(BLOCKS)
RFREE_MAX = HBMAX * H    # free elems per R partition at the largest block
RPAD = RFREE_MAX + 256   # slack so j-shifted SBUF reads stay in-tile
XPAD = HP * H + 8   # per-(i,c) elems in x, padded so shifted reads stay in-bounds

_NC_CACHE = None


def _build_nc():
    # detect_race_conditions=False: the sim race detector resolves SBUF APs to
    # a flat base+partition*row_bytes address model, which false-positives on
    # concurrently-accessed partition-sliced tiles (e.g. the j-shift copies
    # that read partitions 0-31 of R while writing partitions 32j..32j+31).
    nc = bacc.Bacc("TRN2", target_bir_lowering=False, detect_race_conditions=False)
    # x is host-pre-shuffled to [i, c, patch-row h * 512 + m]: x[i, c, h*512+m]
    # = image[c, 4h+i, m].  Each (i, c, 16-patch-row block) is then a single
    # CONTIGUOUS 16 KB run in HBM -> line-rate load descriptors, and the
    # j-shifted variant of a block is the same run offset by 2j bytes (the
    # +8-elem pad keeps the last block's shifted read in-bounds).
    x = nc.dram_tensor("x", [PATCH, C, XPAD], BF16, kind="ExternalInput")
    s = nc.dram_tensor("s", [CHUNKS, 128, 128], BF16, kind="ExternalInput")
    bias = nc.dram_tensor("bias", [128, 128], F32, kind="ExternalInput")
    y = nc.dram_tensor("y", [C * EMBED, HP, HP], F32, kind="ExternalOutput")

    y_v = y.rearrange("ch h w -> ch (h w)")  # [768, 16384]
    # partition (i c), free = patch-row h * 512 + column m: row 4h+i
    x_ic = x.rearrange("i c f -> (i c) f")  # [32, XPAD]

    with tile.TileContext(nc) as tc:
        with (
            tc.tile_pool(name="const", bufs=1) as const_pool,
            tc.tile_pool(name="rin", bufs=2) as r_pool,
            tc.tile_pool(name="stage", bufs=6) as stage_pool,
            tc.tile_pool(name="psum", bufs=2, space="PSUM") as psum_pool,
        ):
            # Pad so every subsequent tile is 512 B-aligned: the framework's
            # const-scalar region ends at +128 B, and SDMA's sub-512B write
            # path does RMW on 512 B granules — cross-tensor granule sharing
            # between concurrent DMA writers would corrupt data.
            _align_pad = const_pool.tile([128, 96], F32, tag="align_pad")
            # Stationary block-diag weights: s_sb[p, chunk*128 + m], bf16,
            # padded to 2048 B/partition.
            s_sb = const_pool.tile([128, 1024], BF16)
            nc.sync.dma_start(
                out=s_sb[:, : CHUNKS * 128].rearrange("p (k m) -> p k m", k=CHUNKS),
                in_=s.rearrange("k p m -> p k m"),
            )
            # Bias: bias_sb[p, chunk] (512 B/partition)
            bias_sb = const_pool.tile([128, 128], F32)
            nc.sync.dma_start(out=bias_sb[:], in_=bias[:])

            hp0 = 0
            evict_flip = 0
            for hb in BLOCKS:
                rfree = hb * H
                win = hb // PATCH  # 512-column windows in this block

                # R: partition p = 32j + 8i + c holds, for each of the block's
                # hb patch-rows hl, image row 4*(hp0+hl)+i of channel c
                # shifted left by j (free pos hl*512 + m = x[c, row, m+j]).
                R = r_pool.tile([128, RPAD], BF16)
                # j=0 and j=1 slices come straight from HBM: 32-partition DMAs
                # with ONE contiguous descriptor per partition (the j=1
                # source run is just offset by 2 bytes).
                for j in range(2):
                    nc.sync.dma_start(
                        out=R[32 * j : 32 * j + 32, 0:rfree],
                        in_=x_ic[:, hp0 * H + j : hp0 * H + j + rfree],
                    )
                # j=2 / j=3 are DVE cross-quadrant copies of j=0 / j=1 shifted
                # by 2 elements (4 bytes, so the 2x/4x DVE streaming modes can
                # engage).  At nch=32 the DVE output crossbar can route bank 0
                # to any quadrant, so [0:32]->[64:96] etc. is a single copy.
                for j in (2, 3):
                    nc.vector.tensor_copy(
                        out=R[32 * j : 32 * j + 32, 0:rfree],
                        in_=R[32 * (j - 2) : 32 * (j - 2) + 32, 2 : 2 + rfree],
                    )

                for chunk in range(CHUNKS):
                    lhsT = s_sb[:, chunk * 128 : (chunk + 1) * 128]
                    stg = stage_pool.tile([128, win * 512], F32)
                    # Up-to-4-bank PSUM tiles: <=4 matmuls fill one, ONE wide
                    # eviction drains it (amortizes the ~0.5us/instr ACT/DVE
                    # overhead over 2048 elements).
                    for g0 in range(0, win, 4):
                        gw = min(4, win - g0)
                        ps = psum_pool.tile([128, gw * 512], F32, tag="ps")
                        for wg in range(gw):
                            w = g0 + wg
                            rhs = R[:, w * 2048 : (w + 1) * 2048 : PATCH]
                            nc.tensor.matmul(
                                ps[:, wg * 512 : (wg + 1) * 512], lhsT, rhs,
                                start=True, stop=True,
                            )
                        out_sl = stg[:, g0 * 512 : (g0 + gw) * 512]
                        evict_flip ^= 1
                        if evict_flip:
                            nc.scalar.activation(
                                out_sl,
                                ps[:],
                                mybir.ActivationFunctionType.Identity,
                                bias=bias_sb[:, chunk : chunk + 1],
                            )
                        else:
                            nc.vector.tensor_scalar_add(
                                out_sl, ps[:], bias_sb[:, chunk : chunk + 1]
                            )
                    # stg partition p -> y channel 128*chunk + p (affine!)
                    # Single full-width 128-partition DMA on the ACT
                    # HWDGE ring so loads (SP ring) and stores don't
                    # serialize on one descriptor ring.
                    nc.scalar.dma_start(
                        out=y_v[
                            128 * chunk : 128 * (chunk + 1),
                            hp0 * HP : (hp0 + hb) * HP,
                        ],
                        in_=stg[:],
                    )
                hp0 += hb
    nc.compile()
    return nc


def _get_nc():
    global _NC_CACHE
    if _NC_CACHE is None:
        _NC_CACHE = _build_nc()
    return _NC_CACHE


def _host_prep(W, b):
    # Stationary chunk t computes GLOBAL output channels g = 128t + m
    # (m = psum partition).  g maps to input channel c = g//96 and conv
    # output channel oc = g%96, so psum partition <-> y channel is affine
    # and the store DMA is a full-width 128-partition transfer.
    # K index k = 32j + 8i + c:  S[t, k, m] = W[oc(g), 0, i, j]
    W2 = np.ascontiguousarray(W, dtype=np.float32).reshape(EMBED, PATCH, PATCH)
    S = np.zeros((CHUNKS, 128, 128), np.float32)
    m = np.arange(128)
    for t in range(CHUNKS):
        g = 128 * t + m
        c = g // EMBED
        oc = g % EMBED
        for i in range(PATCH):
            for j in range(PATCH):
                S[t][32 * j + 8 * i + c, m] = W2[oc, i, j]
    b = np.asarray(b, dtype=np.float32)
    # bias_pad[p, t] = b[(128t+p) % 96]  (padded to [128, 128])
    bias_pad = np.zeros((128, 128), np.float32)
    for t in range(CHUNKS):
        bias_pad[:, t] = b[(128 * t + m) % EMBED]
    return S.astype(ml_dtypes.bfloat16), bias_pad


def _prep_inputs(rgb, hs, dem, W, b):
    x16 = np.empty((N_CORES, C, H, H), ml_dtypes.bfloat16)
    x16[:, :3] = np.asarray(rgb)
    x16[:, 3:7] = np.asarray(hs)
    x16[:, 7:] = np.asarray(dem)
    # [core, c, 4h+i, m] -> [core, i, c, h*512+m], pad each (i,c) run to XPAD
    xs = np.zeros((N_CORES, PATCH, C, XPAD), ml_dtypes.bfloat16)
    xs[..., : HP * H] = (
        x16.reshape(N_CORES, C, HP, PATCH, H)
        .transpose(0, 3, 1, 2, 4)
        .reshape(N_CORES, PATCH, C, HP * H)
    )
    S, bias_mat = _host_prep(W, b)
    return [
        {"x": xs[core], "s": S, "bias": bias_mat} for core in range(N_CORES)
    ]


def _timing_setup(inputs):
    """Build (nc, in_maps) exactly as kernel() would — for test.py --time."""
    in_maps = _prep_inputs(
        inputs["rgb"], inputs["hs"], inputs["dem"], inputs["W"], inputs["b"]
    )
    return _get_nc(), in_maps


def kernel(rgb, hs, dem, W, b):
    in_maps = _prep_inputs(rgb, hs, dem, W, b)
    nc = _get_nc()
    res = run_bass_kernel_spmd(nc, in_maps, list(range(N_CORES)))
    return np.stack([res.results[core]["y"] for core in range(N_CORES)], axis=0)


# revision 19
# speedup vs baseline: 3.8572x; 1.0363x over previous
"""ChannelPatchEmbed kernel for Trainium2 (8 NeuronCores, batch-parallel).

Computation: concat 8 single-feature channels -> each 512x512 image goes
through the SAME 1->96 conv (4x4 patches, stride 4) + bias.
Output: [8, 768, 128, 128] f32.

Strategy per core (1 batch sample per core):
  - GEMM formulation: K = (j, i, c) = 4*4*8 = 128 on the contraction
    partitions, block-diagonal stationary S (6 chunks of 16 output
    channels x 8 input channels = 128 M) -> one K=128 matmul yields 128
    output channels per 512-column pass.
  - bf16: inputs and W are host-cast to bf16 (tolerance 2e-2, bf16
    error ~4e-4); PE runs at full rate, PSUM accumulates f32.
  - Input rows are loaded from HBM exactly ONCE per block as a single
    32-partition DMA (partition (i,c) holds rows r0+4hl+i); the three
    j-shifted copies the matmul layout needs are built on-chip with
    SBUF->SBUF DMAs (32 x 16KB descriptors, cheap) instead of re-reading
    HBM 4x in 2KB packets like the old version.
  - Bias is fused into the PSUM->SBUF eviction (ACT/DVE alternating).
"""

import sys

import numpy as np

if "/opt/trn_rl_repo" not in sys.path:
    sys.path.insert(0, "/opt/trn_rl_repo")

import ml_dtypes

import concourse.bacc as bacc
import concourse.mybir as mybir
import concourse.tile as tile
from concourse.bass_utils import run_bass_kernel_spmd

F32 = mybir.dt.float32
BF16 = mybir.dt.bfloat16

N_CORES = 8
C = 8            # input channels per sample (3 rgb + 4 hs + 1 dem)
H = 512          # image height/width
PATCH = 4
HP = H // PATCH  # 128 patches per side
EMBED = 96
CHUNKS = 6       # 96*8 = 768 output channels in chunks of 128
# patch-rows per block: small first blocks shorten the load->copy->matmul
# ramp before the first stores flow; 32-row blocks give 16 KB store
# descriptors for the bulk.
BLOCKS = [8, 24, 32, 32, 16, 8, 8]
HBMAX = max(BLOCKS)
RFREE_MAX = HBMAX * H    # free elems per R partition at the largest block
RPAD = RFREE_MAX + 256   # slack so j-shifted SBUF reads stay in-tile
XPAD = HP * H + 8   # per-(i,c) elems in x, padded so shifted reads stay in-bounds

_NC_CACHE = None


def _build_nc():
    # detect_race_conditions=False: the sim race detector resolves SBUF APs to
    # a flat base+partition*row_bytes address model, which false-positives on
    # concurrently-accessed partition-sliced tiles (e.g. the j-shift copies
    # that read partitions 0-31 of R while writing partitions 32j..32j+31).
    nc = bacc.Bacc("TRN2", target_bir_lowering=False, detect_race_conditions=False)
    # x is host-pre-shuffled to [i, c, patch-row h * 512 + m]: x[i, c, h*512+m]
    # = image[c, 4h+i, m].  Each (i, c, 16-patch-row block) is then a single
    # CONTIGUOUS 16 KB run in HBM -> line-rate load descriptors, and the
    # j-shifted variant of a block is the same run offset by 2j bytes (the
    # +8-elem pad keeps the last block's shifted read in-bounds).
    x = nc.dram_tensor("x", [PATCH, C, XPAD], BF16, kind="ExternalInput")
    s = nc.dram_tensor("s", [CHUNKS, 128, 128], BF16, kind="ExternalInput")
    bias = nc.dram_tensor("bias", [128, 128], F32, kind="ExternalInput")
    y = nc.dram_tensor("y", [C * EMBED, HP, HP], F32, kind="ExternalOutput")

    y_v = y.rearrange("ch h w -> ch (h w)")  # [768, 16384]
    # partition (i c), free = patch-row h * 512 + column m: row 4h+i
    x_ic = x.rearrange("i c f -> (i c) f")  # [32, XPAD]

    with tile.TileContext(nc) as tc:
        with (
            tc.tile_pool(name="const", bufs=1) as const_pool,
            tc.tile_pool(name="rin", bufs=2) as r_pool,
            tc.tile_pool(name="stage", bufs=6) as stage_pool,
            tc.tile_pool(name="psum", bufs=2, space="PSUM") as psum_pool,
        ):
            # Pad so every subsequent tile is 512 B-aligned: the framework's
            # const-scalar region ends at +128 B, and SDMA's sub-512B write
            # path does RMW on 512 B granules — cross-tensor granule sharing
            # between concurrent DMA writers would corrupt data.
            _align_pad = const_pool.tile([128, 96], F32, tag="align_pad")
            # Stationary block-diag weights: s_sb[p, chunk*128 + m], bf16,
            # padded to 2048 B/partition.
            s_sb = const_pool.tile([128, 1024], BF16)
            nc.sync.dma_start(
                out=s_sb[:, : CHUNKS * 128].rearrange("p (k m) -> p k m", k=CHUNKS),
                in_=s.rearrange("k p m -> p k m"),
            )
            # Bias: bias_sb[p, chunk] (512 B/partition)
            bias_sb = const_pool.tile([128, 128], F32)
            nc.sync.dma_start(out=bias_sb[:], in_=bias[:])

            hp0 = 0
            evict_flip = 0
            for hb in BLOCKS:
                rfree = hb * H
                win = hb // PATCH  # 512-column windows in this block

                # R: partition p = 32j + 8i + c holds, for each of the block's
                # hb patch-rows hl, image row 4*(hp0+hl)+i of channel c
                # shifted left by j (free pos hl*512 + m = x[c, row, m+j]).
                R = r_pool.tile([128, RPAD], BF16)
                # j=0 and j=1 slices come straight from HBM: 32-partition DMAs
                # with ONE contiguous descriptor per partition (the j=1
                # source run is just offset by 2 bytes).
                for j in range(2):
                    nc.sync.dma_start(
                        out=R[32 * j : 32 * j + 32, 0:rfree],
                        in_=x_ic[:, hp0 * H + j : hp0 * H + j + rfree],
                    )
                # j=2 / j=3 are DVE cross-quadrant copies of j=0 / j=1 shifted
                # by 2 elements (4 bytes, so the 2x/4x DVE streaming modes can
                # engage).  At nch=32 the DVE output crossbar can route bank 0
                # to any quadrant, so [0:32]->[64:96] etc. is a single copy.
                for j in (2, 3):
                    nc.vector.tensor_copy(
                        out=R[32 * j : 32 * j + 32, 0:rfree],
                        in_=R[32 * (j - 2) : 32 * (j - 2) + 32, 2 : 2 + rfree],
                    )

                for chunk in range(CHUNKS):
                    lhsT = s_sb[:, chunk * 128 : (chunk + 1) * 128]
                    stg = stage_pool.tile([128, win * 512], F32)
                    # Up-to-4-bank PSUM tiles: <=4 matmuls fill one, ONE wide
                    # eviction drains it (amortizes the ~0.5us/instr ACT/DVE
                    # overhead over 2048 elements).
                    for g0 in range(0, win, 4):
                        gw = min(4, win - g0)
                        ps = psum_pool.tile([128, gw * 512], F32, tag="ps")
                        for wg in range(gw):
                            w = g0 + wg
                            rhs = R[:, w * 2048 : (w + 1) * 2048 : PATCH]
                            nc.tensor.matmul(
                                ps[:, wg * 512 : (wg + 1) * 512], lhsT, rhs,
                                start=True, stop=True,
                            )
                        out_sl = stg[:, g0 * 512 : (g0 + gw) * 512]
                        evict_flip ^= 1
                        if evict_flip:
                            nc.scalar.activation(
                                out_sl,
                                ps[:],
                                mybir.ActivationFunctionType.Identity,
                                bias=bias_sb[:, chunk : chunk + 1],
                            )
                        else:
                            nc.vector.tensor_scalar_add(
                                out_sl, ps[:], bias_sb[:, chunk : chunk + 1]
                            )
                    # stg partition p -> y channel 128*chunk + p (affine!)
                    # Single full-width 128-partition DMA on the ACT
                    # HWDGE ring so loads (SP ring) and stores don't
                    # serialize on one descriptor ring.
                    nc.scalar.dma_start(
                        out=y_v[
                            128 * chunk : 128 * (chunk + 1),
                            hp0 * HP : (hp0 + hb) * HP,
                        ],
                        in_=stg[:],
                    )
                hp0 += hb
    nc.compile()
    return nc


def _get_nc():
    global _NC_CACHE
    if _NC_CACHE is None:
        _NC_CACHE = _build_nc()
    return _NC_CACHE


def _host_prep(W, b):
    # Stationary chunk t computes GLOBAL output channels g = 128t + m
    # (m = psum partition).  g maps to input channel c = g//96 and conv
    # output channel oc = g%96, so psum partition <-> y channel is affine
    # and the store DMA is a full-width 128-partition transfer.
    # K index k = 32j + 8i + c:  S[t, k, m] = W[oc(g), 0, i, j]
    W2 = np.ascontiguousarray(W, dtype=np.float32).reshape(EMBED, PATCH, PATCH)
    S = np.zeros((CHUNKS, 128, 128), np.float32)
    m = np.arange(128)
    for t in range(CHUNKS):
        g = 128 * t + m
        c = g // EMBED
        oc = g % EMBED
        for i in range(PATCH):
            for j in range(PATCH):
                S[t][32 * j + 8 * i + c, m] = W2[oc, i, j]
    b = np.asarray(b, dtype=np.float32)
    # bias_pad[p, t] = b[(128t+p) % 96]  (padded to [128, 128])
    bias_pad = np.zeros((128, 128), np.float32)
    for t in range(CHUNKS):
        bias_pad[:, t] = b[(128 * t + m) % EMBED]
    return S.astype(ml_dtypes.bfloat16), bias_pad


def _prep_inputs(rgb, hs, dem, W, b):
    x16 = np.empty((N_CORES, C, H, H), ml_dtypes.bfloat16)
    x16[:, :3] = np.asarray(rgb)
    x16[:, 3:7] = np.asarray(hs)
    x16[:, 7:] = np.asarray(dem)
    # [core, c, 4h+i, m] -> [core, i, c, h*512+m], pad each (i,c) run to XPAD
    xs = np.zeros((N_CORES, PATCH, C, XPAD), ml_dtypes.bfloat16)
    xs[..., : HP * H] = (
        x16.reshape(N_CORES, C, HP, PATCH, H)
        .transpose(0, 3, 1, 2, 4)
        .reshape(N_CORES, PATCH, C, HP * H)
    )
    S, bias_mat = _host_prep(W, b)
    return [
        {"x": xs[core], "s": S, "bias": bias_mat} for core in range(N_CORES)
    ]


def _timing_setup(inputs):
    """Build (nc, in_maps) exactly as kernel() would — for test.py --time."""
    in_maps = _prep_inputs(
        inputs["rgb"], inputs["hs"], inputs["dem"], inputs["W"], inputs["b"]
    )
    return _get_nc(), in_maps


def kernel(rgb, hs, dem, W, b):
    in_maps = _prep_inputs(rgb, hs, dem, W, b)
    nc = _get_nc()
    res = run_bass_kernel_spmd(nc, in_maps, list(range(N_CORES)))
    return np.stack([res.results[core]["y"] for core in range(N_CORES)], axis=0)


# revision 20
# speedup vs baseline: 4.0496x; 1.0499x over previous
"""ChannelPatchEmbed kernel for Trainium2 (8 NeuronCores, batch-parallel).

Computation: concat 8 single-feature channels -> each 512x512 image goes
through the SAME 1->96 conv (4x4 patches, stride 4) + bias.
Output: [8, 768, 128, 128] f32.

Strategy per core (1 batch sample per core):
  - GEMM formulation: K = (j, i, c) = 4*4*8 = 128 on the contraction
    partitions, block-diagonal stationary S (6 chunks of 16 output
    channels x 8 input channels = 128 M) -> one K=128 matmul yields 128
    output channels per 512-column pass.
  - bf16: inputs and W are host-cast to bf16 (tolerance 2e-2, bf16
    error ~4e-4); PE runs at full rate, PSUM accumulates f32.
  - Input rows are loaded from HBM exactly ONCE per block as a single
    32-partition DMA (partition (i,c) holds rows r0+4hl+i); the three
    j-shifted copies the matmul layout needs are built on-chip with
    SBUF->SBUF DMAs (32 x 16KB descriptors, cheap) instead of re-reading
    HBM 4x in 2KB packets like the old version.
  - Bias is fused into the PSUM->SBUF eviction (ACT/DVE alternating).
"""

import sys

import numpy as np

if "/opt/trn_rl_repo" not in sys.path:
    sys.path.insert(0, "/opt/trn_rl_repo")

import ml_dtypes

import concourse.bacc as bacc
import concourse.mybir as mybir
import concourse.tile as tile
from concourse.bass_utils import run_bass_kernel_spmd

F32 = mybir.dt.float32
BF16 = mybir.dt.bfloat16

N_CORES = 8
C = 8            # input channels per sample (3 rgb + 4 hs + 1 dem)
H = 512          # image height/width
PATCH = 4
HP = H // PATCH  # 128 patches per side
EMBED = 96
CHUNKS = 6       # 96*8 = 768 output channels in chunks of 128
# patch-rows per block: small first blocks shorten the load->copy->matmul
# ramp before the first stores flow; 32-row blocks give 16 KB store
# descriptors for the bulk.
BLOCKS = [8, 24, 32, 32, 32]
HBMAX = max(BLOCKS)
RFREE_MAX = HBMAX * H    # free elems per R partition at the largest block
RPAD = RFREE_MAX + 256   # slack so j-shifted SBUF reads stay in-tile
XPAD = HP * H + 8   # per-(i,c) elems in x, padded so shifted reads stay in-bounds

_NC_CACHE = None


def _build_nc():
    # detect_race_conditions=False: the sim race detector resolves SBUF APs to
    # a flat base+partition*row_bytes address model, which false-positives on
    # concurrently-accessed partition-sliced tiles (e.g. the j-shift copies
    # that read partitions 0-31 of R while writing partitions 32j..32j+31).
    nc = bacc.Bacc("TRN2", target_bir_lowering=False, detect_race_conditions=False)
    # x is host-pre-shuffled to [i, c, patch-row h * 512 + m]: x[i, c, h*512+m]
    # = image[c, 4h+i, m].  Each (i, c, 16-patch-row block) is then a single
    # CONTIGUOUS 16 KB run in HBM -> line-rate load descriptors, and the
    # j-shifted variant of a block is the same run offset by 2j bytes (the
    # +8-elem pad keeps the last block's shifted read in-bounds).
    x = nc.dram_tensor("x", [PATCH, C, XPAD], BF16, kind="ExternalInput")
    s = nc.dram_tensor("s", [CHUNKS, 128, 128], BF16, kind="ExternalInput")
    bias = nc.dram_tensor("bias", [128, 128], F32, kind="ExternalInput")
    y = nc.dram_tensor("y", [C * EMBED, HP, HP], F32, kind="ExternalOutput")

    y_v = y.rearrange("ch h w -> ch (h w)")  # [768, 16384]
    # partition (i c), free = patch-row h * 512 + column m: row 4h+i
    x_ic = x.rearrange("i c f -> (i c) f")  # [32, XPAD]

    with tile.TileContext(nc) as tc:
        with (
            tc.tile_pool(name="const", bufs=1) as const_pool,
            tc.tile_pool(name="rin", bufs=2) as r_pool,
            tc.tile_pool(name="stage", bufs=4) as stage_pool,
            tc.tile_pool(name="psum", bufs=2, space="PSUM") as psum_pool,
        ):
            # Pad so every subsequent tile is 512 B-aligned: the framework's
            # const-scalar region ends at +128 B, and SDMA's sub-512B write
            # path does RMW on 512 B granules — cross-tensor granule sharing
            # between concurrent DMA writers would corrupt data.
            _align_pad = const_pool.tile([128, 96], F32, tag="align_pad")
            # Stationary block-diag weights: s_sb[p, chunk*128 + m], bf16,
            # padded to 2048 B/partition.
            s_sb = const_pool.tile([128, 1024], BF16)
            nc.sync.dma_start(
                out=s_sb[:, : CHUNKS * 128].rearrange("p (k m) -> p k m", k=CHUNKS),
                in_=s.rearrange("k p m -> p k m"),
            )
            # Bias: bias_sb[p, chunk] (512 B/partition)
            bias_sb = const_pool.tile([128, 128], F32)
            nc.sync.dma_start(out=bias_sb[:], in_=bias[:])

            hp0 = 0
            evict_flip = 0
            for hb in BLOCKS:
                rfree = hb * H
                win = hb // PATCH  # 512-column windows in this block

                # R: partition p = 32j + 8i + c holds, for each of the block's
                # hb patch-rows hl, image row 4*(hp0+hl)+i of channel c
                # shifted left by j (free pos hl*512 + m = x[c, row, m+j]).
                R = r_pool.tile([128, RPAD], BF16)
                # j=0 and j=1 slices come straight from HBM: 32-partition DMAs
                # with ONE contiguous descriptor per partition (the j=1
                # source run is just offset by 2 bytes).
                for j in range(2):
                    nc.sync.dma_start(
                        out=R[32 * j : 32 * j + 32, 0:rfree],
                        in_=x_ic[:, hp0 * H + j : hp0 * H + j + rfree],
                    )
                # j=2 / j=3 are DVE cross-quadrant copies of j=0 / j=1 shifted
                # by 2 elements (4 bytes, so the 2x/4x DVE streaming modes can
                # engage).  At nch=32 the DVE output crossbar can route bank 0
                # to any quadrant, so [0:32]->[64:96] etc. is a single copy.
                for j in (2, 3):
                    nc.vector.tensor_copy(
                        out=R[32 * j : 32 * j + 32, 0:rfree],
                        in_=R[32 * (j - 2) : 32 * (j - 2) + 32, 2 : 2 + rfree],
                    )

                for chunk in range(CHUNKS):
                    lhsT = s_sb[:, chunk * 128 : (chunk + 1) * 128]
                    stg = stage_pool.tile([128, win * 512], F32)
                    # Up-to-4-bank PSUM tiles: <=4 matmuls fill one, ONE wide
                    # eviction drains it (amortizes the ~0.5us/instr ACT/DVE
                    # overhead over 2048 elements).
                    for g0 in range(0, win, 4):
                        gw = min(4, win - g0)
                        ps = psum_pool.tile([128, gw * 512], F32, tag="ps")
                        for wg in range(gw):
                            w = g0 + wg
                            rhs = R[:, w * 2048 : (w + 1) * 2048 : PATCH]
                            nc.tensor.matmul(
                                ps[:, wg * 512 : (wg + 1) * 512], lhsT, rhs,
                                start=True, stop=True,
                            )
                        out_sl = stg[:, g0 * 512 : (g0 + gw) * 512]
                        evict_flip ^= 1
                        if evict_flip:
                            nc.scalar.activation(
                                out_sl,
                                ps[:],
                                mybir.ActivationFunctionType.Identity,
                                bias=bias_sb[:, chunk : chunk + 1],
                            )
                        else:
                            nc.vector.tensor_scalar_add(
                                out_sl, ps[:], bias_sb[:, chunk : chunk + 1]
                            )
                    # stg partition p -> y channel 128*chunk + p (affine!)
                    # Single full-width 128-partition DMA on the ACT
                    # HWDGE ring so loads (SP ring) and stores don't
                    # serialize on one descriptor ring.
                    nc.scalar.dma_start(
                        out=y_v[
                            128 * chunk : 128 * (chunk + 1),
                            hp0 * HP : (hp0 + hb) * HP,
                        ],
                        in_=stg[:],
                    )
                hp0 += hb
    nc.compile()
    return nc


def _get_nc():
    global _NC_CACHE
    if _NC_CACHE is None:
        _NC_CACHE = _build_nc()
    return _NC_CACHE


def _host_prep(W, b):
    # Stationary chunk t computes GLOBAL output channels g = 128t + m
    # (m = psum partition).  g maps to input channel c = g//96 and conv
    # output channel oc = g%96, so psum partition <-> y channel is affine
    # and the store DMA is a full-width 128-partition transfer.
    # K index k = 32j + 8i + c:  S[t, k, m] = W[oc(g), 0, i, j]
    W2 = np.ascontiguousarray(W, dtype=np.float32).reshape(EMBED, PATCH, PATCH)
    S = np.zeros((CHUNKS, 128, 128), np.float32)
    m = np.arange(128)
    for t in range(CHUNKS):
        g = 128 * t + m
        c = g // EMBED
        oc = g % EMBED
        for i in range(PATCH):
            for j in range(PATCH):
                S[t][32 * j + 8 * i + c, m] = W2[oc, i, j]
    b = np.asarray(b, dtype=np.float32)
    # bias_pad[p, t] = b[(128t+p) % 96]  (padded to [128, 128])
    bias_pad = np.zeros((128, 128), np.float32)
    for t in range(CHUNKS):
        bias_pad[:, t] = b[(128 * t + m) % EMBED]
    return S.astype(ml_dtypes.bfloat16), bias_pad


def _prep_inputs(rgb, hs, dem, W, b):
    x16 = np.empty((N_CORES, C, H, H), ml_dtypes.bfloat16)
    x16[:, :3] = np.asarray(rgb)
    x16[:, 3:7] = np.asarray(hs)
    x16[:, 7:] = np.asarray(dem)
    # [core, c, 4h+i, m] -> [core, i, c, h*512+m], pad each (i,c) run to XPAD
    xs = np.zeros((N_CORES, PATCH, C, XPAD), ml_dtypes.bfloat16)
    xs[..., : HP * H] = (
        x16.reshape(N_CORES, C, HP, PATCH, H)
        .transpose(0, 3, 1, 2, 4)
        .reshape(N_CORES, PATCH, C, HP * H)
    )
    S, bias_mat = _host_prep(W, b)
    return [
        {"x": xs[core], "s": S, "bias": bias_mat} for core in range(N_CORES)
    ]


def _timing_setup(inputs):
    """Build (nc, in_maps) exactly as kernel() would — for test.py --time."""
    in_maps = _prep_inputs(
        inputs["rgb"], inputs["hs"], inputs["dem"], inputs["W"], inputs["b"]
    )
    return _get_nc(), in_maps


def kernel(rgb, hs, dem, W, b):
    in_maps = _prep_inputs(rgb, hs, dem, W, b)
    nc = _get_nc()
    res = run_bass_kernel_spmd(nc, in_maps, list(range(N_CORES)))
    return np.stack([res.results[core]["y"] for core in range(N_CORES)], axis=0)
